# revision 17
# baseline (speedup 1.0000x reference)
"""EWConv (GNN message passing) Trainium2 kernel, v2.

out = feat @ W_self.T + b_self + agg, where
  agg[d] = (1/max(deg_d,1)) * sum_{e: dst_e=d} exp(-w_e / wsum_d) * (feat[src_e] @ W_pool.T + b_pool)

Linearity lets us aggregate raw features first:
  agg = (A @ feat) @ W_pool.T + rowsum(A) * b_pool,   A[d, s] = sum_e c_e,
  c_e = exp(-w_e / wsum_{dst_e}) / max(deg_{dst_e}, 1)

Sharding: destination nodes are dealt (degree-sorted, round-robin by group of
128) across 8 cores; each core owns its incoming edges. No collectives.

Layout: group of 128 destination nodes at degree level j uses K_j slots per
node, n_j = 128//K_j nodes per 128-slot column, C_j columns. The host expands
feat rows per edge slot into a dense bf16 stream (no gather on device), and
expands per-slot edge weights / per-node inverse degrees into a "wide"
(column, node-within-column) layout so every on-device elementwise op is a
plain contiguous DVE/ACT op. The per-destination segment sum is one PE matmul
per column with the coefficient matrix c3w as rhs (output is S^T: feature on
partitions, node on free dim), so the epilogue
  outT = WpT^T-style matmuls + rank-2 bias matmul
needs no transposes at all.
"""

import math
import os

import numpy as np

P = 128
NC = 8
F = 128


# ---------------------------------------------------------------- host side


def _schedule(dst_np, src_np, efeat_np, N, E):
    """Build the global SPMD schedule + per-core slot tables."""
    deg = np.bincount(dst_np, minlength=N).astype(np.int64)
    order = np.argsort(-deg, kind="stable")  # node ids, degree descending
    L = math.ceil(N / (P * NC))              # levels (groups per core)
    Ntot = L * NC * P
    nodes = np.full(Ntot, -1, dtype=np.int64)
    nodes[:N] = order

    gids = np.arange(Ntot) // P
    core_of_slotpos = gids % NC
    level_of_slotpos = gids // NC
    rank_of_slotpos = level_of_slotpos * P + (np.arange(Ntot) % P)
    core_of = np.empty(N, dtype=np.int64)
    rank_of = np.empty(N, dtype=np.int64)
    valid = nodes >= 0
    core_of[nodes[valid]] = core_of_slotpos[valid]
    rank_of[nodes[valid]] = rank_of_slotpos[valid]

    # per-level K (global max over the 8 cores at that level)
    deg_pad = np.zeros(Ntot, dtype=np.int64)
    deg_pad[valid] = deg[nodes[valid]]
    deg_lvl = deg_pad.reshape(L, NC * P)
    K = np.maximum(4, deg_lvl.max(axis=1))
    n = P // K                      # nodes per column
    C = -(-P // n)                  # columns per group
    col_base = np.concatenate([[0], np.cumsum(C)])
    CTOT = int(col_base[-1])

    # rank -> (p_base, col) within a core
    r = np.arange(L * P)
    jlv = r // P
    q = r % P
    cc = q // n[jlv]
    jj = q % n[jlv]
    rank_pbase = jj * K[jlv]
    rank_col = col_base[jlv] + cc

    # per-core edge tables
    w = efeat_np.reshape(-1).astype(np.float32)
    e_core = core_of[dst_np]
    cores = []
    for c in range(NC):
        sel = np.nonzero(e_core == c)[0]
        er = rank_of[dst_np[sel]]
        o = np.lexsort((src_np[sel], er))
        sel = sel[o]
        er = er[o]
        starts = np.nonzero(np.r_[True, er[1:] != er[:-1]])[0]
        counts = np.diff(np.r_[starts, len(er)])
        k = np.arange(len(er)) - np.repeat(starts, counts)
        p_e = rank_pbase[er] + k
        col_e = rank_col[er]
        assert (k < K[er // P]).all()
        cores.append(dict(eidx=sel, p=p_e, col=col_e,
                          dst=dst_np[sel]))

    # wide (cc, jj) layout offsets and K-runs
    WjW = (C * n).astype(np.int64)          # wide width per group
    goffW = np.concatenate([[0], np.cumsum(WjW)])
    NSW = int(goffW[-1])
    runs = []                               # (K, nj, j0, j1) consecutive eq-K
    j = 0
    while j < L:
        j2 = j
        while j2 < L and K[j2] == K[j]:
            j2 += 1
        runs.append((int(K[j]), int(n[j]), j, j2))
        j2, j = j2, j2
    kdist = []                              # distinct K in run order
    for Kv, nj, _, _ in runs:
        if not kdist or kdist[-1][0] != Kv:
            kdist.append((Kv, nj))

    deg_f = np.maximum(deg, 1).astype(np.float32)
    invdeg = 1.0 / deg_f

    return dict(
        L=L, K=K, n=n, C=C, col_base=col_base, CTOT=CTOT,
        WjW=WjW, goffW=goffW, NSW=NSW, runs=runs, kdist=kdist,
        cores=cores, nodes=nodes, w=w, invdeg=invdeg,
    )


def _build_masks(sch):
    import ml_dtypes
    bf = ml_dtypes.bfloat16
    kdist = sch["kdist"]
    nK = len(kdist)
    njs = [nj for _, nj in kdist]
    koff = np.concatenate([[0], np.cumsum(njs)])
    maskCK = np.zeros((P, int(koff[-1])), dtype=bf)
    maskTK = np.zeros((32, nK * P), dtype=bf)
    p = np.arange(P)
    for i, (Kv, nj) in enumerate(kdist):
        jj_of_p = p // Kv
        ok = jj_of_p < nj
        maskCK[p[ok], koff[i] + jj_of_p[ok]] = 1.0
        maskTK[jj_of_p[ok], i * P + p[ok]] = 1.0
    kidx_of_run = []
    ki = -1
    for Kv, nj, _, _ in sch["runs"]:
        if ki < 0 or kdist[ki][0] != Kv:
            ki += 1
        kidx_of_run.append(ki)
    return maskCK, maskTK, koff, kidx_of_run


def _core_arrays(sch, feat_bf, feat_f8, src_np, c):
    import ml_dtypes
    bf = ml_dtypes.bfloat16
    L, CTOT, NSW = sch["L"], sch["CTOT"], sch["NSW"]
    K, n, C, col_base = sch["K"], sch["n"], sch["C"], sch["col_base"]
    goffW = sch["goffW"]
    ed = sch["cores"][c]

    src_slot = np.full((P, CTOT), -1, dtype=np.int64)
    src_slot[ed["p"], ed["col"]] = src_np[ed["eidx"]]
    vmask = src_slot >= 0
    sfeat = feat_f8[src_slot.clip(min=0)]           # [P, CTOT, F]
    sfeat[~vmask] = 0
    sfeat = np.ascontiguousarray(sfeat.reshape(P, CTOT * F))

    wgrid = np.zeros((P, CTOT), dtype=np.float32)
    wgrid[ed["p"], ed["col"]] = sch["w"][ed["eidx"]]

    nl = sch["nodes"].reshape(L, NC, P)[:, c, :].reshape(-1)
    nlv = nl >= 0

    ivdN = np.zeros((P, CTOT), dtype=np.float32)
    ivdN[ed["p"], ed["col"]] = sch["invdeg"][ed["dst"]]

    fperm = feat_bf[nl.clip(min=0)].astype(np.float32)
    fperm[~nlv] = 0
    fpermT = np.ascontiguousarray(fperm.T.astype(bf))  # [F, L*P]
    wpadN = wgrid.astype(bf)
    return sfeat, wpadN, ivdN.astype(bf), fpermT, nl


# ---------------------------------------------------------------- device side


def _build_bass(sch, koff, kidx_of_run, has_bias):
    import concourse.bass as bass  # noqa: F401
    import concourse.bacc as bacc
    import concourse.tile as tile
    from concourse import mybir

    L, K, n, C = sch["L"], sch["K"], sch["n"], sch["C"]
    col_base, CTOT, NSW = sch["col_base"], sch["CTOT"], sch["NSW"]
    goffW, runs = sch["goffW"], sch["runs"]
    Cmax = int(C.max())
    nK = len(sch["kdist"])
    f32 = mybir.dt.float32
    bf16 = mybir.dt.bfloat16
    f8 = mybir.dt.float8e4
    Alu = mybir.AluOpType
    nKC = int(koff[-1])

    nc = bacc.Bacc("TRN2", target_bir_lowering=False, debug=False,
                   num_devices=NC)
    d_sfeat = nc.dram_tensor("sfeat", [P, CTOT * F], f8, kind="ExternalInput")
    d_wpadN = nc.dram_tensor("wpadN", [P, CTOT], bf16, kind="ExternalInput")
    d_ivdN = nc.dram_tensor("ivdN", [P, CTOT], bf16, kind="ExternalInput")
    d_fpermT = nc.dram_tensor("fpermT", [F, L * P], bf16, kind="ExternalInput")
    d_maskCK = nc.dram_tensor("maskCK", [P, nKC], bf16, kind="ExternalInput")
    d_maskTK = nc.dram_tensor("maskTK", [32, nK * P], bf16, kind="ExternalInput")
    d_WpT = nc.dram_tensor("WpTb", [F, F], bf16, kind="ExternalInput")
    d_WsT = nc.dram_tensor("WsTb", [F, F], bf16, kind="ExternalInput")
    if has_bias:
        d_bias = nc.dram_tensor("biasT2", [2, F], bf16, kind="ExternalInput")
    d_outT = nc.dram_tensor("outT", [F, L * P], f32, kind="ExternalOutput")

    CHUNK = 512

    with tile.TileContext(nc) as tc:
        with (
            tc.tile_pool(name="const", bufs=1) as cp,
            tc.tile_pool(name="grp", bufs=6) as gp,
            tc.tile_pool(name="sfp", bufs=12) as ga,
            tc.tile_pool(name="epi", bufs=3) as ep,
            tc.tile_pool(name="ps_grid", bufs=2, space="PSUM") as pgrid,
            tc.tile_pool(name="ps_exp", bufs=2, space="PSUM") as pexp,
            tc.tile_pool(name="ps_s", bufs=2, space="PSUM") as pS,
            tc.tile_pool(name="ps_o", bufs=2, space="PSUM") as pO,
        ):
            wpadN = cp.tile([P, CTOT], bf16)
            nc.sync.dma_start(wpadN[:], d_wpadN[:])
            ivdN = cp.tile([P, CTOT], bf16)
            nc.sync.dma_start(ivdN[:], d_ivdN[:])
            fpermT = cp.tile([F, L * P], bf16)
            nc.gpsimd.dma_start(fpermT[:], d_fpermT[:])
            maskCK = cp.tile([P, nKC], bf16)
            nc.sync.dma_start(maskCK[:], d_maskCK[:])
            maskTK = cp.tile([32, nK * P], bf16)
            nc.sync.dma_start(maskTK[:], d_maskTK[:])
            WpT = cp.tile([F, F], bf16)
            nc.sync.dma_start(WpT[:], d_WpT[:])
            WsT = cp.tile([F, F], bf16)
            nc.sync.dma_start(WsT[:], d_WsT[:])
            if has_bias:
                biasT2 = cp.tile([2, F], bf16)
                nc.sync.dma_start(biasT2[:], d_bias[:])
                onesc = cp.tile([P, 1], bf16)
                nc.vector.memset(onesc[:], 1.0)
                csum_sb = cp.tile([2, L * P], bf16)
                nc.vector.memset(csum_sb[1:2, :], 1.0)

            # resident sfeat: whole fp8 stream in SBUF, 4 chunk tiles
            # (separate tiles so group segsums only depend on their chunk)
            bnds = list(range(0, L, 2)) + [L]
            NCH = len(bnds) - 1
            sfc = []
            for k in range(NCH):
                ca = int(col_base[bnds[k]])
                cbb = int(col_base[bnds[k + 1]])
                t = cp.tile([P, cbb - ca, F], f8, name=f"sfc{k}")
                nc.sync.dma_start(t[:], d_sfeat[:, ca * F : cbb * F])
                sfc.append((t, ca))
            chunk_of = np.searchsorted(
                np.asarray(bnds), np.arange(L), side="right") - 1

            RW = 704  # max run width in the wide layout
            assert all(
                int(goffW[j1]) - int(goffW[j0]) <= RW for _, _, j0, j1 in runs
            )

            def a1(ri):
                """Narrow per-node recip for run ri -> widened recw tile."""
                Kv, nj, j0, j1 = runs[ri]
                ki = kidx_of_run[ri]
                ko = int(koff[ki])
                cbase = int(col_base[j0])
                Crun = int(col_base[j1]) - cbase
                grid = pgrid.tile([32, 128], f32, tag="grid")
                nc.tensor.matmul(
                    grid[:nj, :Crun],
                    maskCK[:, ko : ko + nj],
                    wpadN[:, cbase : cbase + Crun],
                    start=True, stop=True,
                )
                gsb = gp.tile([32, 128], f32, tag="gsb")
                nc.vector.tensor_scalar_max(
                    gsb[:nj, :Crun], grid[:nj, :Crun], 1e-20
                )
                recf = gp.tile([32, 128], f32, tag="recf")
                nc.vector.reciprocal(recf[:nj, :Crun], gsb[:nj, :Crun])
                recw = gp.tile([32, 192], bf16, tag="recw")
                nc.vector.tensor_copy(recw[:nj, :Crun], recf[:nj, :Crun])
                return recw

            def a2(ri, recw):
                """Per-slot coefficients c3w for run ri (local offsets).

                All elementwise work is on the narrow [128, Crun] per-column
                layout; the wide (cc, jj) rhs is produced by one
                double-broadcast multiply with the 0/1 node-select pattern
                (maskCK), which also zeroes pad slots via ivdN=0.
                """
                Kv, nj, j0, j1 = runs[ri]
                ki = kidx_of_run[ri]
                ko = int(koff[ki])
                cbase = int(col_base[j0])
                Crun = int(col_base[j1]) - cbase
                W = int(goffW[j1]) - int(goffW[j0])
                expd = pexp.tile([P, CHUNK], f32)
                nc.tensor.matmul(
                    expd[:, :Crun],
                    maskTK[:nj, ki * P : (ki + 1) * P],
                    recw[:nj, :Crun],
                    start=True, stop=True,
                )
                expb = gp.tile([P, 192], bf16, tag="expb")
                nc.vector.tensor_copy(expb[:, :Crun], expd[:, :Crun])
                t_sb = gp.tile([P, 192], bf16, tag="tsb")
                nc.vector.tensor_tensor(
                    t_sb[:, :Crun], wpadN[:, cbase : cbase + Crun],
                    expb[:, :Crun], Alu.mult,
                )
                nc.scalar.activation(
                    t_sb[:, :Crun], t_sb[:, :Crun],
                    mybir.ActivationFunctionType.Exp, scale=-1.0,
                )
                c3n = gp.tile([P, 192], bf16, tag="c3n")
                nc.vector.tensor_tensor(
                    c3n[:, :Crun], t_sb[:, :Crun],
                    ivdN[:, cbase : cbase + Crun], Alu.mult,
                )
                c3r = gp.tile([P, RW], bf16, tag="c3r")
                pstep = c3r[:].ap[0][0]
                wide = bass.AP(
                    c3r[:].tensor, c3r[:].offset,
                    [[pstep, P], [nj, Crun], [1, nj]],
                )
                nbc = bass.AP(
                    c3n[:].tensor, c3n[:].offset,
                    [[c3n[:].ap[0][0], P], [1, Crun], [0, nj]],
                )
                mbc = bass.AP(
                    maskCK[:].tensor, maskCK[:].offset + ko,
                    [[maskCK[:].ap[0][0], P], [0, Crun], [1, nj]],
                )
                nc.vector.tensor_tensor(wide, nbc, mbc, Alu.mult)
                return c3r

            EB = 4  # groups per epilogue batch
            st4 = {}          # batch -> [128, EB*128] PSUM tile

            def seg_one(j, c3r, base):
                """Segment-sum matmuls for group j into its ST4 quarter."""
                Cj = int(C[j])
                nj = int(n[j])
                gW = int(goffW[j]) - base
                cb = int(col_base[j])
                jb = j // EB
                if jb not in st4:
                    st4[jb] = pS.tile([P, EB * P], f32, tag="st4",
                                      name="st4")
                q = (j % EB) * P
                sf, ca = sfc[int(chunk_of[j])]
                coff = cb - ca
                for cc in range(Cj):
                    nje = min(nj, P - cc * nj)
                    nc.tensor.matmul(
                        st4[jb][:, q + cc * nj : q + cc * nj + nje],
                        sf[:, coff + cc, :],
                        c3r[:, gW + cc * nj : gW + cc * nj + nje],
                        start=True, stop=True,
                    )
                if has_bias:
                    W = int(Cj * nj)
                    csp = pgrid.tile([1, CHUNK], f32, tag="csum")
                    nc.tensor.matmul(
                        csp[:, :W], onesc[:], c3r[:, gW : gW + W],
                        start=True, stop=True,
                    )
                    nc.vector.tensor_copy(
                        csum_sb[0:1, j * P : (j + 1) * P], csp[:, :P]
                    )

            def epi_batch(jb):
                """Epilogue for groups [jb*EB, ...): one wide matmul set."""
                j0b = jb * EB
                nb = (min(L, j0b + EB) - j0b) * P
                ST = st4.pop(jb)
                ST_sb = ep.tile([P, EB * P], bf16, tag="ST")
                nc.scalar.activation(
                    ST_sb[:, :nb], ST[:, :nb],
                    mybir.ActivationFunctionType.Copy,
                )
                OUT = pO.tile([P, EB * P], f32, tag="out4", name="out4")
                nc.tensor.matmul(
                    OUT[:, :nb], WpT[:], ST_sb[:, :nb],
                    start=True, stop=False,
                )
                nc.tensor.matmul(
                    OUT[:, :nb], WsT[:],
                    fpermT[:, j0b * P : j0b * P + nb],
                    start=False, stop=not has_bias,
                )
                if has_bias:
                    nc.tensor.matmul(
                        OUT[:, :nb], biasT2[:],
                        csum_sb[:, j0b * P : j0b * P + nb],
                        start=False, stop=True,
                    )
                o_sb = ep.tile([P, EB * P], f32, tag="o_sb")
                nc.scalar.activation(
                    o_sb[:, :nb], OUT[:, :nb],
                    mybir.ActivationFunctionType.Copy,
                )
                nc.gpsimd.dma_start(
                    d_outT[:, j0b * P : j0b * P + nb], o_sb[:, :nb]
                )

            # software pipeline: a1 leads by 2 runs, a2 by 1 run; the
            # epilogue batch for groups [4b, 4b+4) is emitted once the
            # segsums of group 4b+5 are in the queue, so the in-order PE
            # queue never waits on Scalar-engine PSUM drains.
            NR = len(runs)
            rw_tiles = {}
            c3_tiles = {}
            for r0 in range(min(3, NR)):
                rw_tiles[r0] = a1(r0)
            for r0 in range(min(2, NR)):
                c3_tiles[r0] = a2(r0, rw_tiles.pop(r0))
            done_b = 0
            j_seen = 0
            for ri in range(NR):
                if ri + 2 < NR:
                    c3_tiles[ri + 2] = a2(ri + 2, rw_tiles.pop(ri + 2))
                if ri + 3 < NR:
                    rw_tiles[ri + 3] = a1(ri + 3)
                c3r = c3_tiles.pop(ri)
                _, _, j0, j1 = runs[ri]
                base = int(goffW[j0])
                for j in range(j0, j1):
                    seg_one(j, c3r, base)
                    j_seen = j
                    while (done_b + 1) * EB + 1 < j_seen:
                        epi_batch(done_b)
                        done_b += 1
            while done_b * EB < L:
                epi_batch(done_b)
                done_b += 1

    nc.compile()
    return nc


# ---------------------------------------------------------------- entry point

_CACHE = {}
LAST_EXEC_NS = None


def kernel(feat, efeat, src, dst, W_pool, b_pool, W_self, b_self):
    import ml_dtypes
    bf = ml_dtypes.bfloat16

    feat = np.asarray(feat, dtype=np.float32)
    efeat = np.asarray(efeat, dtype=np.float32)
    src_np = np.asarray(src).astype(np.int64)
    dst_np = np.asarray(dst).astype(np.int64)
    N, E = feat.shape[0], src_np.shape[0]

    b_pool = np.asarray(b_pool, dtype=np.float32)
    b_self = np.asarray(b_self, dtype=np.float32)
    has_bias = bool(np.any(b_pool) or np.any(b_self))

    sch = _schedule(dst_np, src_np, efeat, N, E)
    maskCK, maskTK, koff, kidx_of_run = _build_masks(sch)

    key = (N, E, sch["CTOT"], sch["NSW"], tuple(sch["K"]), has_bias)
    if key not in _CACHE:
        _CACHE[key] = _build_bass(sch, koff, kidx_of_run, has_bias)
    nc = _CACHE[key]

    feat_bf = feat.astype(bf)
    feat_f8 = feat.astype(ml_dtypes.float8_e4m3)
    WpTb = np.ascontiguousarray(np.asarray(W_pool, np.float32).T.astype(bf))
    WsTb = np.ascontiguousarray(np.asarray(W_self, np.float32).T.astype(bf))

    in_maps = []
    nls = []
    for c in range(NC):
        sfeat, wpadN, ivdN, fpermT, nl = _core_arrays(
            sch, feat_bf, feat_f8, src_np, c)
        m = {
            "sfeat": sfeat, "wpadN": wpadN, "ivdN": ivdN,
            "fpermT": fpermT,
            "maskCK": maskCK, "maskTK": maskTK, "WpTb": WpTb, "WsTb": WsTb,
        }
        if has_bias:
            m["biasT2"] = np.stack([b_pool, b_self]).astype(bf)
        in_maps.append(m)
        nls.append(nl)

    from concourse.bass_utils import run_bass_kernel_spmd

    trace = False
    if os.environ.get("KERNEL_TRACE"):
        try:
            import sys as _sys
            import types as _types
            if "antenv.axon_hooks" not in _sys.modules:
                _m = _types.ModuleType("antenv.axon_hooks")
                _h = [None]
                _m.set_axon_ntff_profile_hook = lambda h: _h.__setitem__(0, h)
                _m.get_axon_ntff_profile_hook = lambda: _h[0]
                _sys.modules["antenv.axon_hooks"] = _m
                import antenv
                antenv.axon_hooks = _m
                _sys.path.insert(0, "/root/.axon_site")
                from trn_agent_boot.trn_boot import _ntff_profile_via_ctypes
                _m.set_axon_ntff_profile_hook(
                    _ntff_profile_via_ctypes("/opt/axon/libaxon_pjrt.so"))
            trace = True
        except Exception:
            trace = False

    res = run_bass_kernel_spmd(nc, in_maps, core_ids=list(range(NC)),
                               trace=trace)
    global LAST_EXEC_NS
    LAST_EXEC_NS = res.exec_time_ns

    out = np.empty((N, F), dtype=np.float32)
    for c in range(NC):
        opT = res.results[c]["outT"]        # [F, L*P]
        nl = nls[c]
        v = nl >= 0
        out[nl[v]] = opT[:, v].T
    return out


# revision 18
# speedup vs baseline: 1.0445x; 1.0445x over previous
"""EWConv (GNN message passing) Trainium2 kernel, v2.

out = feat @ W_self.T + b_self + agg, where
  agg[d] = (1/max(deg_d,1)) * sum_{e: dst_e=d} exp(-w_e / wsum_d) * (feat[src_e] @ W_pool.T + b_pool)

Linearity lets us aggregate raw features first:
  agg = (A @ feat) @ W_pool.T + rowsum(A) * b_pool,   A[d, s] = sum_e c_e,
  c_e = exp(-w_e / wsum_{dst_e}) / max(deg_{dst_e}, 1)

Sharding: destination nodes are dealt (degree-sorted, round-robin by group of
128) across 8 cores; each core owns its incoming edges. No collectives.

Layout: group of 128 destination nodes at degree level j uses K_j slots per
node, n_j = 128//K_j nodes per 128-slot column, C_j columns. The host expands
feat rows per edge slot into a dense bf16 stream (no gather on device), and
expands per-slot edge weights / per-node inverse degrees into a "wide"
(column, node-within-column) layout so every on-device elementwise op is a
plain contiguous DVE/ACT op. The per-destination segment sum is one PE matmul
per column with the coefficient matrix c3w as rhs (output is S^T: feature on
partitions, node on free dim), so the epilogue
  outT = WpT^T-style matmuls + rank-2 bias matmul
needs no transposes at all.
"""

import math
import os

import numpy as np

P = 128
NC = 8
F = 128


# ---------------------------------------------------------------- host side


def _schedule(dst_np, src_np, efeat_np, N, E):
    """Build the global SPMD schedule + per-core slot tables."""
    deg = np.bincount(dst_np, minlength=N).astype(np.int64)
    order = np.argsort(-deg, kind="stable")  # node ids, degree descending
    L = math.ceil(N / (P * NC))              # levels (groups per core)
    Ntot = L * NC * P
    nodes = np.full(Ntot, -1, dtype=np.int64)
    nodes[:N] = order

    gids = np.arange(Ntot) // P
    core_of_slotpos = gids % NC
    level_of_slotpos = gids // NC
    rank_of_slotpos = level_of_slotpos * P + (np.arange(Ntot) % P)
    core_of = np.empty(N, dtype=np.int64)
    rank_of = np.empty(N, dtype=np.int64)
    valid = nodes >= 0
    core_of[nodes[valid]] = core_of_slotpos[valid]
    rank_of[nodes[valid]] = rank_of_slotpos[valid]

    # per-level K (global max over the 8 cores at that level)
    deg_pad = np.zeros(Ntot, dtype=np.int64)
    deg_pad[valid] = deg[nodes[valid]]
    deg_lvl = deg_pad.reshape(L, NC * P)
    K = np.maximum(4, deg_lvl.max(axis=1))
    n = P // K                      # nodes per column
    C = -(-P // n)                  # columns per group
    col_base = np.concatenate([[0], np.cumsum(C)])
    CTOT = int(col_base[-1])

    # rank -> (p_base, col) within a core
    r = np.arange(L * P)
    jlv = r // P
    q = r % P
    cc = q // n[jlv]
    jj = q % n[jlv]
    rank_pbase = jj * K[jlv]
    rank_col = col_base[jlv] + cc

    # per-core edge tables
    w = efeat_np.reshape(-1).astype(np.float32)
    e_core = core_of[dst_np]
    cores = []
    for c in range(NC):
        sel = np.nonzero(e_core == c)[0]
        er = rank_of[dst_np[sel]]
        o = np.lexsort((src_np[sel], er))
        sel = sel[o]
        er = er[o]
        starts = np.nonzero(np.r_[True, er[1:] != er[:-1]])[0]
        counts = np.diff(np.r_[starts, len(er)])
        k = np.arange(len(er)) - np.repeat(starts, counts)
        p_e = rank_pbase[er] + k
        col_e = rank_col[er]
        assert (k < K[er // P]).all()
        cores.append(dict(eidx=sel, p=p_e, col=col_e,
                          dst=dst_np[sel]))

    # wide (cc, jj) layout offsets and K-runs
    WjW = (C * n).astype(np.int64)          # wide width per group
    goffW = np.concatenate([[0], np.cumsum(WjW)])
    NSW = int(goffW[-1])
    runs = []                               # (K, nj, j0, j1) consecutive eq-K
    j = 0
    while j < L:
        j2 = j
        while j2 < L and K[j2] == K[j]:
            j2 += 1
        runs.append((int(K[j]), int(n[j]), j, j2))
        j2, j = j2, j2
    kdist = []                              # distinct K in run order
    for Kv, nj, _, _ in runs:
        if not kdist or kdist[-1][0] != Kv:
            kdist.append((Kv, nj))

    deg_f = np.maximum(deg, 1).astype(np.float32)
    invdeg = 1.0 / deg_f

    return dict(
        L=L, K=K, n=n, C=C, col_base=col_base, CTOT=CTOT,
        WjW=WjW, goffW=goffW, NSW=NSW, runs=runs, kdist=kdist,
        cores=cores, nodes=nodes, w=w, invdeg=invdeg,
    )


def _build_masks(sch):
    import ml_dtypes
    bf = ml_dtypes.bfloat16
    kdist = sch["kdist"]
    nK = len(kdist)
    njs = [nj for _, nj in kdist]
    koff = np.concatenate([[0], np.cumsum(njs)])
    maskCK = np.zeros((P, int(koff[-1])), dtype=bf)
    maskTK = np.zeros((32, nK * P), dtype=bf)
    p = np.arange(P)
    for i, (Kv, nj) in enumerate(kdist):
        jj_of_p = p // Kv
        ok = jj_of_p < nj
        maskCK[p[ok], koff[i] + jj_of_p[ok]] = 1.0
        maskTK[jj_of_p[ok], i * P + p[ok]] = 1.0
    kidx_of_run = []
    ki = -1
    for Kv, nj, _, _ in sch["runs"]:
        if ki < 0 or kdist[ki][0] != Kv:
            ki += 1
        kidx_of_run.append(ki)
    return maskCK, maskTK, koff, kidx_of_run


def _core_arrays(sch, feat_bf, feat_f8, src_np, c):
    import ml_dtypes
    bf = ml_dtypes.bfloat16
    L, CTOT, NSW = sch["L"], sch["CTOT"], sch["NSW"]
    K, n, C, col_base = sch["K"], sch["n"], sch["C"], sch["col_base"]
    goffW = sch["goffW"]
    ed = sch["cores"][c]

    src_slot = np.full((P, CTOT), -1, dtype=np.int64)
    src_slot[ed["p"], ed["col"]] = src_np[ed["eidx"]]
    vmask = src_slot >= 0
    sfeat = feat_f8[src_slot.clip(min=0)]           # [P, CTOT, F]
    sfeat[~vmask] = 0
    sfeat = np.ascontiguousarray(sfeat.reshape(P, CTOT * F))

    wgrid = np.zeros((P, CTOT), dtype=np.float32)
    wgrid[ed["p"], ed["col"]] = sch["w"][ed["eidx"]]

    nl = sch["nodes"].reshape(L, NC, P)[:, c, :].reshape(-1)
    nlv = nl >= 0

    ivdN = np.zeros((P, CTOT), dtype=np.float32)
    ivdN[ed["p"], ed["col"]] = sch["invdeg"][ed["dst"]]

    fperm = feat_bf[nl.clip(min=0)].astype(np.float32)
    fperm[~nlv] = 0
    fpermT = np.ascontiguousarray(fperm.T.astype(bf))  # [F, L*P]
    wpadN = wgrid.astype(bf)
    return sfeat, wpadN, ivdN.astype(bf), fpermT, nl


# ---------------------------------------------------------------- device side


def _build_bass(sch, koff, kidx_of_run, has_bias):
    import concourse.bass as bass  # noqa: F401
    import concourse.bacc as bacc
    import concourse.tile as tile
    from concourse import mybir

    L, K, n, C = sch["L"], sch["K"], sch["n"], sch["C"]
    col_base, CTOT, NSW = sch["col_base"], sch["CTOT"], sch["NSW"]
    goffW, runs = sch["goffW"], sch["runs"]
    Cmax = int(C.max())
    nK = len(sch["kdist"])
    f32 = mybir.dt.float32
    bf16 = mybir.dt.bfloat16
    f8 = mybir.dt.float8e4
    Alu = mybir.AluOpType
    nKC = int(koff[-1])

    nc = bacc.Bacc("TRN2", target_bir_lowering=False, debug=False,
                   num_devices=NC)
    d_sfeat = nc.dram_tensor("sfeat", [P, CTOT * F], f8, kind="ExternalInput")
    d_wpadN = nc.dram_tensor("wpadN", [P, CTOT], bf16, kind="ExternalInput")
    d_ivdN = nc.dram_tensor("ivdN", [P, CTOT], bf16, kind="ExternalInput")
    d_fpermT = nc.dram_tensor("fpermT", [F, L * P], bf16, kind="ExternalInput")
    d_maskCK = nc.dram_tensor("maskCK", [P, nKC], bf16, kind="ExternalInput")
    d_maskTK = nc.dram_tensor("maskTK", [32, nK * P], bf16, kind="ExternalInput")
    d_WpT = nc.dram_tensor("WpTb", [F, F], bf16, kind="ExternalInput")
    d_WsT = nc.dram_tensor("WsTb", [F, F], bf16, kind="ExternalInput")
    if has_bias:
        d_bias = nc.dram_tensor("biasT2", [2, F], bf16, kind="ExternalInput")
    d_outT = nc.dram_tensor("outT", [F, L * P], f32, kind="ExternalOutput")

    CHUNK = 512

    with tile.TileContext(nc) as tc:
        with (
            tc.tile_pool(name="const", bufs=1) as cp,
            tc.tile_pool(name="grp", bufs=6) as gp,
            tc.tile_pool(name="sfp", bufs=12) as ga,
            tc.tile_pool(name="epi", bufs=3) as ep,
            tc.tile_pool(name="ps_grid", bufs=2, space="PSUM") as pgrid,
            tc.tile_pool(name="ps_exp", bufs=2, space="PSUM") as pexp,
            tc.tile_pool(name="ps_s", bufs=2, space="PSUM") as pS,
            tc.tile_pool(name="ps_o", bufs=2, space="PSUM") as pO,
        ):
            wpadN = cp.tile([P, CTOT], bf16)
            nc.sync.dma_start(wpadN[:], d_wpadN[:])
            ivdN = cp.tile([P, CTOT], bf16)
            nc.sync.dma_start(ivdN[:], d_ivdN[:])
            fpermT = cp.tile([F, L * P], bf16)
            nc.sync.dma_start(fpermT[:], d_fpermT[:])
            maskCK = cp.tile([P, nKC], bf16)
            nc.sync.dma_start(maskCK[:], d_maskCK[:])
            maskTK = cp.tile([32, nK * P], bf16)
            nc.sync.dma_start(maskTK[:], d_maskTK[:])
            WpT = cp.tile([F, F], bf16)
            nc.sync.dma_start(WpT[:], d_WpT[:])
            WsT = cp.tile([F, F], bf16)
            nc.sync.dma_start(WsT[:], d_WsT[:])
            if has_bias:
                biasT2 = cp.tile([2, F], bf16)
                nc.sync.dma_start(biasT2[:], d_bias[:])
                onesc = cp.tile([P, 1], bf16)
                nc.vector.memset(onesc[:], 1.0)
                csum_sb = cp.tile([2, L * P], bf16)
                nc.vector.memset(csum_sb[1:2, :], 1.0)

            RW = 704  # max run width in the wide layout
            assert all(
                int(goffW[j1]) - int(goffW[j0]) <= RW for _, _, j0, j1 in runs
            )

            def a1(ri):
                """Narrow per-node recip for run ri -> widened recw tile."""
                Kv, nj, j0, j1 = runs[ri]
                ki = kidx_of_run[ri]
                ko = int(koff[ki])
                cbase = int(col_base[j0])
                Crun = int(col_base[j1]) - cbase
                grid = pgrid.tile([32, 128], f32, tag="grid")
                nc.tensor.matmul(
                    grid[:nj, :Crun],
                    maskCK[:, ko : ko + nj],
                    wpadN[:, cbase : cbase + Crun],
                    start=True, stop=True,
                )
                gsb = gp.tile([32, 128], f32, tag="gsb")
                nc.vector.tensor_scalar_max(
                    gsb[:nj, :Crun], grid[:nj, :Crun], 1e-20
                )
                recf = gp.tile([32, 128], f32, tag="recf")
                nc.vector.reciprocal(recf[:nj, :Crun], gsb[:nj, :Crun])
                recw = gp.tile([32, 192], bf16, tag="recw")
                nc.vector.tensor_copy(recw[:nj, :Crun], recf[:nj, :Crun])
                return recw

            def a2(ri, recw):
                """Per-slot coefficients c3w for run ri (local offsets).

                All elementwise work is on the narrow [128, Crun] per-column
                layout; the wide (cc, jj) rhs is produced by one
                double-broadcast multiply with the 0/1 node-select pattern
                (maskCK), which also zeroes pad slots via ivdN=0.
                """
                Kv, nj, j0, j1 = runs[ri]
                ki = kidx_of_run[ri]
                ko = int(koff[ki])
                cbase = int(col_base[j0])
                Crun = int(col_base[j1]) - cbase
                W = int(goffW[j1]) - int(goffW[j0])
                expd = pexp.tile([P, CHUNK], f32)
                nc.tensor.matmul(
                    expd[:, :Crun],
                    maskTK[:nj, ki * P : (ki + 1) * P],
                    recw[:nj, :Crun],
                    start=True, stop=True,
                )
                expb = gp.tile([P, 192], bf16, tag="expb")
                nc.vector.tensor_copy(expb[:, :Crun], expd[:, :Crun])
                t_sb = gp.tile([P, 192], bf16, tag="tsb")
                nc.vector.tensor_tensor(
                    t_sb[:, :Crun], wpadN[:, cbase : cbase + Crun],
                    expb[:, :Crun], Alu.mult,
                )
                nc.scalar.activation(
                    t_sb[:, :Crun], t_sb[:, :Crun],
                    mybir.ActivationFunctionType.Exp, scale=-1.0,
                )
                c3n = gp.tile([P, 192], bf16, tag="c3n")
                nc.vector.tensor_tensor(
                    c3n[:, :Crun], t_sb[:, :Crun],
                    ivdN[:, cbase : cbase + Crun], Alu.mult,
                )
                c3r = gp.tile([P, RW], bf16, tag="c3r")
                pstep = c3r[:].ap[0][0]
                wide = bass.AP(
                    c3r[:].tensor, c3r[:].offset,
                    [[pstep, P], [nj, Crun], [1, nj]],
                )
                nbc = bass.AP(
                    c3n[:].tensor, c3n[:].offset,
                    [[c3n[:].ap[0][0], P], [1, Crun], [0, nj]],
                )
                mbc = bass.AP(
                    maskCK[:].tensor, maskCK[:].offset + ko,
                    [[maskCK[:].ap[0][0], P], [0, Crun], [1, nj]],
                )
                nc.vector.tensor_tensor(wide, nbc, mbc, Alu.mult)
                return c3r

            EB = 4  # groups per epilogue batch
            st4 = {}          # batch -> [128, EB*128] PSUM tile

            def seg_one(j, c3r, base):
                """Segment-sum matmuls for group j into its ST4 quarter."""
                Cj = int(C[j])
                nj = int(n[j])
                gW = int(goffW[j]) - base
                cb = int(col_base[j])
                jb = j // EB
                if jb not in st4:
                    st4[jb] = pS.tile([P, EB * P], f32, tag="st4",
                                      name="st4")
                q = (j % EB) * P
                sf = ga.tile([P, Cmax, F], f8, tag="sf")
                nc.sync.dma_start(
                    sf[:, :Cj, :], d_sfeat[:, cb * F : (cb + Cj) * F]
                )
                for cc in range(Cj):
                    nje = min(nj, P - cc * nj)
                    nc.tensor.matmul(
                        st4[jb][:, q + cc * nj : q + cc * nj + nje],
                        sf[:, cc, :],
                        c3r[:, gW + cc * nj : gW + cc * nj + nje],
                        start=True, stop=True,
                    )
                if has_bias:
                    W = int(Cj * nj)
                    csp = pgrid.tile([1, CHUNK], f32, tag="csum")
                    nc.tensor.matmul(
                        csp[:, :W], onesc[:], c3r[:, gW : gW + W],
                        start=True, stop=True,
                    )
                    nc.vector.tensor_copy(
                        csum_sb[0:1, j * P : (j + 1) * P], csp[:, :P]
                    )

            def epi_batch(jb):
                """Epilogue for groups [jb*EB, ...): one wide matmul set."""
                j0b = jb * EB
                nb = (min(L, j0b + EB) - j0b) * P
                ST = st4.pop(jb)
                ST_sb = ep.tile([P, EB * P], bf16, tag="ST")
                nc.scalar.activation(
                    ST_sb[:, :nb], ST[:, :nb],
                    mybir.ActivationFunctionType.Copy,
                )
                OUT = pO.tile([P, EB * P], f32, tag="out4", name="out4")
                nc.tensor.matmul(
                    OUT[:, :nb], WpT[:], ST_sb[:, :nb],
                    start=True, stop=False,
                )
                nc.tensor.matmul(
                    OUT[:, :nb], WsT[:],
                    fpermT[:, j0b * P : j0b * P + nb],
                    start=False, stop=not has_bias,
                )
                if has_bias:
                    nc.tensor.matmul(
                        OUT[:, :nb], biasT2[:],
                        csum_sb[:, j0b * P : j0b * P + nb],
                        start=False, stop=True,
                    )
                o_sb = ep.tile([P, EB * P], f32, tag="o_sb")
                nc.scalar.activation(
                    o_sb[:, :nb], OUT[:, :nb],
                    mybir.ActivationFunctionType.Copy,
                )
                nc.gpsimd.dma_start(
                    d_outT[:, j0b * P : j0b * P + nb], o_sb[:, :nb]
                )

            # software pipeline: a1 leads by 2 runs, a2 by 1 run; the
            # epilogue batch for groups [4b, 4b+4) is emitted once the
            # segsums of group 4b+5 are in the queue, so the in-order PE
            # queue never waits on Scalar-engine PSUM drains.
            NR = len(runs)
            rw_tiles = {}
            c3_tiles = {}
            for r0 in range(min(3, NR)):
                rw_tiles[r0] = a1(r0)
            for r0 in range(min(2, NR)):
                c3_tiles[r0] = a2(r0, rw_tiles.pop(r0))
            done_b = 0
            j_seen = 0
            for ri in range(NR):
                if ri + 2 < NR:
                    c3_tiles[ri + 2] = a2(ri + 2, rw_tiles.pop(ri + 2))
                if ri + 3 < NR:
                    rw_tiles[ri + 3] = a1(ri + 3)
                c3r = c3_tiles.pop(ri)
                _, _, j0, j1 = runs[ri]
                base = int(goffW[j0])
                for j in range(j0, j1):
                    seg_one(j, c3r, base)
                    j_seen = j
                    while (done_b + 1) * EB + 1 < j_seen:
                        epi_batch(done_b)
                        done_b += 1
            while done_b * EB < L:
                epi_batch(done_b)
                done_b += 1

    nc.compile()
    return nc


# ---------------------------------------------------------------- entry point

_CACHE = {}
LAST_EXEC_NS = None


def kernel(feat, efeat, src, dst, W_pool, b_pool, W_self, b_self):
    import ml_dtypes
    bf = ml_dtypes.bfloat16

    feat = np.asarray(feat, dtype=np.float32)
    efeat = np.asarray(efeat, dtype=np.float32)
    src_np = np.asarray(src).astype(np.int64)
    dst_np = np.asarray(dst).astype(np.int64)
    N, E = feat.shape[0], src_np.shape[0]

    b_pool = np.asarray(b_pool, dtype=np.float32)
    b_self = np.asarray(b_self, dtype=np.float32)
    has_bias = bool(np.any(b_pool) or np.any(b_self))

    sch = _schedule(dst_np, src_np, efeat, N, E)
    maskCK, maskTK, koff, kidx_of_run = _build_masks(sch)

    key = (N, E, sch["CTOT"], sch["NSW"], tuple(sch["K"]), has_bias)
    if key not in _CACHE:
        _CACHE[key] = _build_bass(sch, koff, kidx_of_run, has_bias)
    nc = _CACHE[key]

    feat_bf = feat.astype(bf)
    feat_f8 = feat.astype(ml_dtypes.float8_e4m3)
    WpTb = np.ascontiguousarray(np.asarray(W_pool, np.float32).T.astype(bf))
    WsTb = np.ascontiguousarray(np.asarray(W_self, np.float32).T.astype(bf))

    in_maps = []
    nls = []
    for c in range(NC):
        sfeat, wpadN, ivdN, fpermT, nl = _core_arrays(
            sch, feat_bf, feat_f8, src_np, c)
        m = {
            "sfeat": sfeat, "wpadN": wpadN, "ivdN": ivdN,
            "fpermT": fpermT,
            "maskCK": maskCK, "maskTK": maskTK, "WpTb": WpTb, "WsTb": WsTb,
        }
        if has_bias:
            m["biasT2"] = np.stack([b_pool, b_self]).astype(bf)
        in_maps.append(m)
        nls.append(nl)

    from concourse.bass_utils import run_bass_kernel_spmd

    trace = False
    if os.environ.get("KERNEL_TRACE"):
        try:
            import sys as _sys
            import types as _types
            if "antenv.axon_hooks" not in _sys.modules:
                _m = _types.ModuleType("antenv.axon_hooks")
                _h = [None]
                _m.set_axon_ntff_profile_hook = lambda h: _h.__setitem__(0, h)
                _m.get_axon_ntff_profile_hook = lambda: _h[0]
                _sys.modules["antenv.axon_hooks"] = _m
                import antenv
                antenv.axon_hooks = _m
                _sys.path.insert(0, "/root/.axon_site")
                from trn_agent_boot.trn_boot import _ntff_profile_via_ctypes
                _m.set_axon_ntff_profile_hook(
                    _ntff_profile_via_ctypes("/opt/axon/libaxon_pjrt.so"))
            trace = True
        except Exception:
            trace = False

    res = run_bass_kernel_spmd(nc, in_maps, core_ids=list(range(NC)),
                               trace=trace)
    global LAST_EXEC_NS
    LAST_EXEC_NS = res.exec_time_ns

    out = np.empty((N, F), dtype=np.float32)
    for c in range(NC):
        opT = res.results[c]["outT"]        # [F, L*P]
        nl = nls[c]
        v = nl >= 0
        out[nl[v]] = opT[:, v].T
    return out
